# revision 1
# baseline (speedup 1.0000x reference)
"""Multi-head attention (B=4, L=2048, D=1024, H=16) on 8 Trainium2 NeuronCores.

Sharding: core c = (batch b = c//2, query-half qh = c%2). Each core computes
all 16 heads for its 1024 query rows against the full 2048 keys/values of its
batch. Fully SPMD, no collectives. Host does transpose/slice prep and
concatenation gather only.

Per-core pipeline (all matmuls fp32r, N=512):
  1. kpT[dh,1024 x L,2048], qpT[dh,1024 x Lq,1024] = W^T-major projections
     vp[L,2048 x dh,1024]  = value projection (L-major)
  2. per head-pair, per 512-col query chunk:
       S^T[j,i] = kpT^T qpT (row-packed pairs, K=64)
       P = exp(S^T) (no max subtraction; scores ~N(0,1), |s| < ~7)
       outT[dh,i] += vp^T P (col-packed pairs), Z[i] += 1^T P (ones-matmul)
       outT /= Z (reciprocal + K=1 broadcast matmul + DVE mul)
  3. out[l,o] = outT^T woT + bo  (streamed from DRAM scratch)
"""

import sys

if "/opt/trn_rl_repo" not in sys.path:
    sys.path.insert(0, "/opt/trn_rl_repo")

import numpy as np

import concourse.bacc as bacc
import concourse.tile as tile
from concourse import mybir
from concourse.bass_utils import run_bass_kernel_spmd

N_CORES = 8
B, L, D = 4, 2048, 1024
NH, DH = 16, 64          # heads, head dim
LQ = L // 2              # query rows per core
F32 = mybir.dt.float32
F32R = mybir.dt.float32r

KC = D // 128            # 8 contraction chunks for projections
MQ = LQ // 128           # 8 l-chunks per core
NJ = L // 128            # 16 key j-chunks
NI = LQ // 512           # 2 query i-chunks of 512
NPAIR = NH // 2          # 8 head pairs
JGROUPS = [3, 3, 3, 3, 3, 1]  # 16 j-chunks split into exp groups


def build_program(dbg=False):
    nc = bacc.Bacc("TRN2", target_bir_lowering=False, debug=False,
                   num_devices=N_CORES)
    with tile.TileContext(nc) as tc:
        _emit(nc, tc, dbg=dbg)
    nc.compile()
    return nc


def _emit(nc, tc, dbg=False):
    from contextlib import ExitStack

    top = ExitStack()
    dram = top.enter_context(tc.tile_pool(name="dram", bufs=1, space="DRAM"))
    xqT = dram.tile([D, LQ], F32R, kind="ExternalInput", name="xqT", uniquify=False)
    xkT = dram.tile([D, L], F32R, kind="ExternalInput", name="xkT", uniquify=False)
    xvT = dram.tile([D, L], F32R, kind="ExternalInput", name="xvT", uniquify=False)
    wqT = dram.tile([D, D], F32R, kind="ExternalInput", name="wqT", uniquify=False)
    wkT = dram.tile([D, D], F32R, kind="ExternalInput", name="wkT", uniquify=False)
    wvT = dram.tile([D, D], F32R, kind="ExternalInput", name="wvT", uniquify=False)
    woT = dram.tile([D, D], F32R, kind="ExternalInput", name="woT", uniquify=False)
    bqc = dram.tile([128, KC], F32, kind="ExternalInput", name="bqc", uniquify=False)
    bkc = dram.tile([128, KC], F32, kind="ExternalInput", name="bkc", uniquify=False)
    bvr = dram.tile([1, D], F32R, kind="ExternalInput", name="bvr", uniquify=False)
    bor = dram.tile([1, D], F32R, kind="ExternalInput", name="bor", uniquify=False)
    c_or = dram.tile([1, 128], F32R, kind="ExternalInput", name="c_or", uniquify=False)
    c_oc = dram.tile([128, 1], F32R, kind="ExternalInput", name="c_oc", uniquify=False)
    c_sel = dram.tile([2, 128], F32R, kind="ExternalInput", name="c_sel", uniquify=False)
    c_vone = dram.tile([128, NH], F32R, kind="ExternalInput", name="c_vone",
                       uniquify=False)
    out = dram.tile([LQ, D], F32, kind="ExternalOutput", name="out", uniquify=False)
    outT_d = dram.tile([D, LQ], F32R, kind="Internal", name="outT_d")
    if dbg:
        qpT_dbg = dram.tile([D, LQ], F32R, kind="ExternalOutput",
                            name="qpT_dbg", uniquify=False)
        kpT_dbg = dram.tile([D, L], F32R, kind="ExternalOutput",
                            name="kpT_dbg", uniquify=False)
        vpa_dbg = dram.tile([NJ * 128, NH * 65], F32R, kind="ExternalOutput",
                            name="vpa_dbg", uniquify=False)
        outT_dbg = dram.tile([D, LQ], F32R, kind="ExternalOutput",
                             name="outT_dbg", uniquify=False)
        e_dbg = dram.tile([128, 1536], F32R, kind="ExternalOutput",
                          name="e_dbg", uniquify=False)
        z_dbg = dram.tile([2, 512], F32, kind="ExternalOutput",
                          name="z_dbg", uniquify=False)

    # persistent SBUF
    pers = top.enter_context(tc.tile_pool(name="pers", bufs=1))
    kpT = [pers.tile([128, L], F32R, name=f"kpT{m}") for m in range(KC)]
    qpT = [pers.tile([128, LQ], F32R, name=f"qpT{m}") for m in range(KC)]
    # vpa: per j-chunk, 16 heads x (64 value cols + 1 ones col) -> Z via PV
    vpa = [pers.tile([128, NH * 65], F32R, name=f"vpa{m}") for m in range(NJ)]
    ones1 = pers.tile([1, 128], F32R, name="ones1")
    ones128 = pers.tile([128, 1], F32R, name="ones128")
    sel2 = pers.tile([2, 128], F32R, name="sel2")
    bq_sb = pers.tile([128, KC], F32, name="bq_sb")
    bk_sb = pers.tile([128, KC], F32, name="bk_sb")
    bv_sb = pers.tile([1, D], F32R, name="bv_sb")
    bo_sb = pers.tile([1, D], F32R, name="bo_sb")

    nc.sync.dma_start(out=ones1[:], in_=c_or[:])
    nc.sync.dma_start(out=ones128[:], in_=c_oc[:])
    nc.sync.dma_start(out=sel2[:], in_=c_sel[:])
    for m in range(NJ):
        nc.sync.dma_start(
            out=vpa[m].rearrange("p (h c) -> p h c", c=65)[:, :, 64:65],
            in_=c_vone.rearrange("p (h o) -> p h o", o=1))
    nc.sync.dma_start(out=bq_sb[:], in_=bqc[:])
    nc.sync.dma_start(out=bk_sb[:], in_=bkc[:])
    nc.sync.dma_start(out=bv_sb[:], in_=bvr[:])
    nc.sync.dma_start(out=bo_sb[:], in_=bor[:])

    xkT_r = xkT.rearrange("(kc p) l -> p kc l", p=128)
    xqT_r = xqT.rearrange("(kc p) l -> p kc l", p=128)
    xvT_r = xvT.rearrange("(kc p) l -> p kc l", p=128)
    wqT_r = wqT.rearrange("(kc p) m -> p kc m", p=128)
    wkT_r = wkT.rearrange("(kc p) m -> p kc m", p=128)
    wvT_r = wvT.rearrange("(kc p) m -> p kc m", p=128)
    woT_r = woT.rearrange("(kc p) m -> p kc m", p=128)
    outT_r = outT_d.rearrange("(kc p) l -> p kc l", p=128)

    # ---- phase 1: projections -------------------------------------------
    with ExitStack() as proj_ctx:
        px = proj_ctx.enter_context(tc.tile_pool(name="px", bufs=2))
        pw = proj_ctx.enter_context(tc.tile_pool(name="pw", bufs=3))
        pp = proj_ctx.enter_context(tc.tile_pool(name="pp", bufs=4, space="PSUM"))

        # kpT / qpT:  psum[dh128, l512] = sum_kc wT[:,kc,m128].T @ xT[:,kc,n512]
        # x streamed in 256-wide half-blocks to fit SBUF (fp32r full-rate >=256)
        for (w_r, x_r, dst, bias, nn) in (
            (wkT_r, xkT_r, kpT, bk_sb, L // 256),
            (wqT_r, xqT_r, qpT, bq_sb, LQ // 256),
        ):
            for n in range(0, nn, 2):
                xa = px.tile([128, KC, 256], F32R, tag="px")
                xb = px.tile([128, KC, 256], F32R, tag="px")
                nc.sync.dma_start(out=xa[:], in_=x_r[:, :, n * 256:(n + 1) * 256])
                nc.sync.dma_start(out=xb[:], in_=x_r[:, :, (n + 1) * 256:(n + 2) * 256])
                for m in range(KC):
                    wb = pw.tile([128, KC, 128], F32R, tag="pw")
                    nc.sync.dma_start(out=wb[:], in_=w_r[:, :, m * 128:(m + 1) * 128])
                    ps = pp.tile([128, 512], F32, tag="pp")
                    # fp32r accumulation groups must not interleave within a
                    # psum bank: run the two 256-wide halves sequentially
                    for kc in range(KC):
                        nc.tensor.matmul(ps[:, 0:256], wb[:, kc, :], xa[:, kc, :],
                                         start=(kc == 0), stop=(kc == KC - 1))
                    for kc in range(KC):
                        nc.tensor.matmul(ps[:, 256:512], wb[:, kc, :], xb[:, kc, :],
                                         start=(kc == 0), stop=(kc == KC - 1))
                    nc.vector.tensor_scalar_add(
                        dst[m][:, n * 256:(n + 2) * 256], ps[:], bias[:, m:m + 1])

        # vp: psum[l128, dh512] = bias + sum_kc xvT[:,kc,m128].T @ wvT[:,kc,n512]
        for n in range(D // 512):
            wa = px.tile([128, KC, 256], F32R, tag="px")
            wb2 = px.tile([128, KC, 256], F32R, tag="px")
            nc.sync.dma_start(out=wa[:], in_=wvT_r[:, :, n * 512:n * 512 + 256])
            nc.sync.dma_start(out=wb2[:], in_=wvT_r[:, :, n * 512 + 256:(n + 1) * 512])
            for m in range(NJ):
                xb2 = pw.tile([128, KC, 128], F32R, tag="pw")
                nc.sync.dma_start(out=xb2[:], in_=xvT_r[:, :, m * 128:(m + 1) * 128])
                ps = pp.tile([128, 512], F32, tag="pp")
                nc.tensor.matmul(ps[:, 0:256], ones1[0:1, :],
                                 bv_sb[0:1, n * 512:n * 512 + 256],
                                 start=True, stop=False)
                for kc in range(KC):
                    nc.tensor.matmul(ps[:, 0:256], xb2[:, kc, :], wa[:, kc, :],
                                     start=False, stop=(kc == KC - 1))
                nc.tensor.matmul(ps[:, 256:512], ones1[0:1, :],
                                 bv_sb[0:1, n * 512 + 256:(n + 1) * 512],
                                 start=True, stop=False)
                for kc in range(KC):
                    nc.tensor.matmul(ps[:, 256:512], xb2[:, kc, :], wb2[:, kc, :],
                                     start=False, stop=(kc == KC - 1))
                nc.vector.tensor_copy(
                    vpa[m].rearrange("p (h c) -> p h c", c=65)
                    [:, 8 * n:8 * (n + 1), 0:64], ps[:])

    if dbg:
        for m in range(KC):
            nc.sync.dma_start(out=qpT_dbg[m * 128:(m + 1) * 128, :], in_=qpT[m][:])
            nc.sync.dma_start(out=kpT_dbg[m * 128:(m + 1) * 128, :], in_=kpT[m][:])
        for m in range(NJ):
            nc.sync.dma_start(out=vpa_dbg[m * 128:(m + 1) * 128, :], in_=vpa[m][:])

    # ---- phase 2: attention ---------------------------------------------
    with ExitStack() as attn_ctx:
        psA = attn_ctx.enter_context(tc.tile_pool(name="psA", bufs=2, space="PSUM"))
        psO = attn_ctx.enter_context(tc.tile_pool(name="psO", bufs=2, space="PSUM"))
        pe = attn_ctx.enter_context(tc.tile_pool(name="pe", bufs=3))
        pst = attn_ctx.enter_context(tc.tile_pool(name="pst", bufs=1))
        prc = attn_ctx.enter_context(tc.tile_pool(name="prc", bufs=1))

        for p in range(NPAIR):
            hA, hB = 2 * p, 2 * p + 1
            for ic in range(NI):
                isl = slice(ic * 512, (ic + 1) * 512)
                # rows 0-63: head output; row 64: Z (from vpa's ones column)
                ozA = psO.tile([65, 512], F32, tag="o")
                ozB = psO.tile([65, 512], F32, tag="o")
                jbase = 0
                for g, gs in enumerate(JGROUPS):
                    sA = psA.tile([128, 1536], F32, tag="s")
                    sB = psA.tile([128, 1536], F32, tag="s")
                    for gg in range(gs):
                        jc = jbase + gg
                        jsl = slice(jc * 128, (jc + 1) * 128)
                        gsl = slice(gg * 512, (gg + 1) * 512)
                        nc.tensor.matmul(sA[:, gsl], kpT[p][0:64, jsl],
                                         qpT[p][0:64, isl],
                                         tile_position=(0, 0))
                        nc.tensor.matmul(sB[:, gsl], kpT[p][64:128, jsl],
                                         qpT[p][64:128, isl],
                                         tile_position=(64, 0))
                    eA = pe.tile([128, 1536], F32R, tag="e")
                    eB = pe.tile([128, 1536], F32R, tag="e")
                    nc.scalar.activation(eA[:, 0:gs * 512], sA[:, 0:gs * 512],
                                         mybir.ActivationFunctionType.Exp)
                    nc.scalar.activation(eB[:, 0:gs * 512], sB[:, 0:gs * 512],
                                         mybir.ActivationFunctionType.Exp)
                    if dbg and p == 0 and ic == 0 and g == 0:
                        nc.sync.dma_start(out=e_dbg[:], in_=eA[:])
                    first = (g == 0)
                    for gg in range(gs):
                        jc = jbase + gg
                        last = (jc == NJ - 1)
                        gsl = slice(gg * 512, (gg + 1) * 512)
                        nc.tensor.matmul(ozA[:, :],
                                         vpa[jc][:, hA * 65:(hA + 1) * 65],
                                         eA[:, gsl],
                                         start=(first and gg == 0), stop=last)
                        nc.tensor.matmul(ozB[:, :],
                                         vpa[jc][:, hB * 65:(hB + 1) * 65],
                                         eB[:, gsl],
                                         start=(first and gg == 0), stop=last)
                    jbase += gs

                # Z rows live at psum partition 64; move to partitions 0/1 of
                # one SBUF tile via (partition-base-aligned) DVE copy + DMA.
                zt = prc.tile([65, 1024], F32, tag="zt")
                nc.vector.tensor_copy(zt[64:65, 0:512], ozA[64:65, :])
                nc.vector.tensor_copy(zt[64:65, 512:1024], ozB[64:65, :])
                zr = prc.tile([2, 512], F32, tag="zr")
                nc.sync.dma_start(out=zr[0:1, :], in_=zt[64:65, 0:512])
                nc.sync.dma_start(out=zr[1:2, :], in_=zt[64:65, 512:1024])
                recip = prc.tile([2, 512], F32R, tag="rc")
                with nc.allow_low_precision(reason="fp32r rounding of 1/Z"):
                    nc.vector.reciprocal(recip[:], zr[:])
                # broadcast 1/Z across 64 partitions: K=2 selector matmuls
                rA = psA.tile([64, 512], F32, tag="s")
                rB = psA.tile([64, 512], F32, tag="s")
                nc.tensor.matmul(rA[:, :], sel2[0:2, 0:64], recip[:])
                nc.tensor.matmul(rB[:, :], sel2[0:2, 64:128], recip[:])
                rsbA = pst.tile([64, 512], F32, tag="rsb")
                rsbB = pst.tile([64, 512], F32, tag="rsb2")
                nc.vector.tensor_copy(rsbA[:], rA[:, :])
                nc.vector.tensor_copy(rsbB[:], rB[:, :])
                stA = pst.tile([64, 512], F32R, tag="st")
                stB = pst.tile([64, 512], F32R, tag="st2")
                nc.vector.tensor_mul(stA[:], ozA[0:64, :], rsbA[:])
                nc.vector.tensor_mul(stB[:], ozB[0:64, :], rsbB[:])
                nc.sync.dma_start(
                    out=outT_d[p * 128:p * 128 + 64, isl], in_=stA[:])
                nc.sync.dma_start(
                    out=outT_d[p * 128 + 64:p * 128 + 128, isl], in_=stB[:])
                if dbg:
                    nc.sync.dma_start(
                        out=outT_dbg[p * 128:p * 128 + 64, isl], in_=stA[:])
                    nc.sync.dma_start(
                        out=outT_dbg[p * 128 + 64:p * 128 + 128, isl],
                        in_=stB[:])
                    if p == 0 and ic == 0:
                        nc.sync.dma_start(out=z_dbg[:], in_=zr[:])

    # ---- phase 3: output projection -------------------------------------
    with ExitStack() as fin_ctx:
        fw = fin_ctx.enter_context(tc.tile_pool(name="fw", bufs=1))
        fo = fin_ctx.enter_context(tc.tile_pool(name="fo", bufs=3))
        fs = fin_ctx.enter_context(tc.tile_pool(name="fs", bufs=3))
        pf = fin_ctx.enter_context(tc.tile_pool(name="pf", bufs=4, space="PSUM"))

        for n in range(D // 512):
            nsl = slice(n * 512, (n + 1) * 512)
            wob = fw.tile([128, KC, 512], F32R, tag="fw")
            nc.sync.dma_start(out=wob[:], in_=woT_r[:, :, nsl])
            for m in range(MQ):
                otb = fo.tile([128, KC, 128], F32R, tag="fo")
                nc.sync.dma_start(out=otb[:],
                                  in_=outT_r[:, :, m * 128:(m + 1) * 128])
                ps = pf.tile([128, 512], F32, tag="pf")
                nc.tensor.matmul(ps[:], ones1[0:1, :], bo_sb[0:1, nsl],
                                 start=True, stop=False)
                for kc in range(KC):
                    nc.tensor.matmul(ps[:], otb[:, kc, :], wob[:, kc, :],
                                     start=False, stop=(kc == KC - 1))
                ost = fs.tile([128, 512], F32, tag="fs")
                nc.scalar.copy(ost[:], ps[:])
                nc.sync.dma_start(out=out[m * 128:(m + 1) * 128, nsl],
                                  in_=ost[:])


_NC_CACHE = None


def _get_program():
    global _NC_CACHE
    if _NC_CACHE is None:
        _NC_CACHE = build_program()
    return _NC_CACHE


def prep_in_maps(q, k, v, w_q, b_q, w_k, b_k, w_v, b_v, w_o, b_o):
    f = np.float32
    q, k, v = (np.asarray(t, f) for t in (q, k, v))
    scale = 1.0 / np.sqrt(DH)
    wqT = np.ascontiguousarray((np.asarray(w_q, f) * scale).T)
    wkT = np.ascontiguousarray(np.asarray(w_k, f).T)
    wvT = np.ascontiguousarray(np.asarray(w_v, f).T)
    woT = np.ascontiguousarray(np.asarray(w_o, f).T)
    bqc = np.ascontiguousarray((np.asarray(b_q, f) * scale).reshape(KC, 128).T)
    bkc = np.ascontiguousarray(np.asarray(b_k, f).reshape(KC, 128).T)
    bvr = np.asarray(b_v, f).reshape(1, D)
    bor = np.asarray(b_o, f).reshape(1, D)
    c_or = np.ones((1, 128), f)
    c_oc = np.ones((128, 1), f)
    c_sel = np.zeros((2, 128), f)
    c_sel[0, 0:64] = 1.0
    c_sel[1, 64:128] = 1.0
    c_vone = np.ones((128, NH), f)
    in_maps = []
    for c in range(N_CORES):
        b, qh = c // 2, c % 2
        kTb = np.ascontiguousarray(k[b].T)
        vTb = np.ascontiguousarray(v[b].T)
        qTb = np.ascontiguousarray(q[b].T[:, qh * LQ:(qh + 1) * LQ])
        in_maps.append({
            "xqT": qTb, "xkT": kTb, "xvT": vTb,
            "wqT": wqT, "wkT": wkT, "wvT": wvT, "woT": woT,
            "bqc": bqc, "bkc": bkc, "bvr": bvr, "bor": bor,
            "c_or": c_or, "c_oc": c_oc, "c_sel": c_sel, "c_vone": c_vone,
        })
    return in_maps


def run(in_maps, trace=False, **kw):
    nc = _get_program()
    return run_bass_kernel_spmd(nc, in_maps, list(range(N_CORES)),
                                trace=trace, **kw)


def kernel(**inputs):
    in_maps = prep_in_maps(**inputs)
    res = run(in_maps)
    out = np.empty((B, L, D), np.float32)
    for c in range(N_CORES):
        b, qh = c // 2, c % 2
        out[b, qh * LQ:(qh + 1) * LQ, :] = res.results[c]["out"]
    return out



# revision 10
# speedup vs baseline: 2.2591x; 2.2591x over previous
"""Multi-head attention (B=4, L=2048, D=1024, H=16) on 8 Trainium2 NeuronCores.

Sharding: core c = (batch b = c//2, query-half qh = c%2). Each core computes
all 16 heads for its 1024 query rows against the full 2048 keys/values of its
batch. Fully SPMD, no collectives.

v2: all-bf16 data path (fp32 psum), software-pipelined attention emission
(S matmuls run two groups ahead of exp/PV so the PE never idles on the
activation engine), k/q projections for pair p+1 interleaved into pair p's
attention stream, deferred batched softmax normalization (no per-block
reciprocal chain), and SBUF-resident activations end-to-end (x, w, outT all
loaded once; no DRAM scratch roundtrip).

Per-core pipeline:
  1. vp[j,16x(64+1)] value projection + ones col (PV also yields Z row 64)
  2. kpT[p][128,2048], qpT[p][128,1024] per head-pair p (pair 0 upfront,
     pairs 1-7 streamed inside the attention loop)
  3. per (pair, 512-query block), 8 groups of 2 key chunks:
       S^T = kpT^T qpT (row-packed head pairs, K=64) -> psum
       P = exp(S^T) bf16 (ACT)  ->  outU += vp^T P, Z += ones^T P (PE)
     Z rows DMA-gathered into zall[32,512] (async, off critical path)
  4. rz = 1/zall (one DVE op), per (pair,ic): 2 selector matmuls broadcast
     rz across 64 partitions, one DVE mul normalizes outU in place
  5. out[l,o] = outU^T woT + bo streamed to DRAM
"""

import sys

if "/opt/trn_rl_repo" not in sys.path:
    sys.path.insert(0, "/opt/trn_rl_repo")

import numpy as np

import concourse.bacc as bacc
import concourse.tile as tile
from concourse import mybir
from concourse.bass_utils import run_bass_kernel_spmd

N_CORES = 8
B, L, D = 4, 2048, 1024
NH, DH = 16, 64          # heads, head dim
LQ = L // 2              # query rows per core
F32 = mybir.dt.float32
F32R = mybir.dt.float32r
BF16 = mybir.dt.bfloat16

KC = D // 128            # 8 contraction chunks for projections
NJ = L // 128            # 16 key j-chunks
NI = LQ // 512           # 2 query i-chunks of 512
NPAIR = NH // 2          # 8 head pairs
EXPF = mybir.ActivationFunctionType.Exp


def build_program():
    nc = bacc.Bacc("TRN2", target_bir_lowering=False, debug=False,
                   num_devices=N_CORES)
    with tile.TileContext(nc) as tc:
        _emit(nc, tc)
    nc.compile()
    return nc


def _emit(nc, tc):
    from contextlib import ExitStack

    top = ExitStack()
    dram = top.enter_context(tc.tile_pool(name="dram", bufs=1, space="DRAM"))

    def din(shape, dt, name):
        return dram.tile(shape, dt, kind="ExternalInput", name=name,
                         uniquify=False)

    xqT = din([D, LQ], BF16, "xqT")
    xkT = din([D, L], BF16, "xkT")
    xvT = din([D, L], BF16, "xvT")
    wqT = din([D, D], BF16, "wqT")
    wkT = din([D, D], BF16, "wkT")
    wvT = din([D, D], BF16, "wvT")
    woT = din([D, D], BF16, "woT")
    bqc = din([128, KC], F32, "bqc")
    bkc = din([128, KC], F32, "bkc")
    bvr = din([1, D], BF16, "bvr")
    bor = din([1, D], BF16, "bor")
    c_or = din([1, 128], BF16, "c_or")
    c_vone = din([128, NH], BF16, "c_vone")
    c_sel = din([64, 64 * 4 * NPAIR], F32R, "c_sel")
    out = dram.tile([LQ, D], F32, kind="ExternalOutput", name="out",
                    uniquify=False)

    # ---- persistent SBUF -------------------------------------------------
    pers = top.enter_context(tc.tile_pool(name="pers", bufs=1))
    kpT = [pers.tile([128, L], BF16, name=f"kpT{m}") for m in range(NPAIR)]
    qpT = [pers.tile([128, LQ], BF16, name=f"qpT{m}") for m in range(NPAIR)]
    # vpa: per j-chunk, 16 heads x (64 value cols + 1 ones col)
    vpa = [pers.tile([128, NH * 65], BF16, name=f"vpa{m}") for m in range(NJ)]
    outU = [pers.tile([128, LQ], BF16, name=f"outU{m}") for m in range(NPAIR)]
    xk_sb = pers.tile([128, KC, L], BF16, name="xk_sb")
    xq_sb = pers.tile([128, KC, LQ], BF16, name="xq_sb")
    wv_sb = pers.tile([128, KC, D], BF16, name="wv_sb")
    ones1 = pers.tile([1, 128], BF16, name="ones1")
    # 64 partitions (rows 32+ pad: zall 1.0, sel 0) for valid 64x64 PE tiling
    sel_sb = pers.tile([64, 64 * 4 * NPAIR], F32R, name="sel_sb")
    zall = pers.tile([64, 512], F32, name="zall")
    nc.vector.memset(zall[4 * NPAIR:64, :], 1.0)
    bq_sb = pers.tile([128, KC], F32, name="bq_sb")
    bk_sb = pers.tile([128, KC], F32, name="bk_sb")
    bv_sb = pers.tile([1, D], BF16, name="bv_sb")
    bo_sb = pers.tile([1, D], BF16, name="bo_sb")

    nc.sync.dma_start(out=ones1[:], in_=c_or[:])
    nc.sync.dma_start(out=sel_sb[:], in_=c_sel[:])
    nc.sync.dma_start(out=bq_sb[:], in_=bqc[:])
    nc.sync.dma_start(out=bk_sb[:], in_=bkc[:])
    nc.sync.dma_start(out=bv_sb[:], in_=bvr[:])
    nc.sync.dma_start(out=bo_sb[:], in_=bor[:])
    for m in range(NJ):
        nc.sync.dma_start(
            out=vpa[m].rearrange("p (h c) -> p h c", c=65)[:, :, 64:65],
            in_=c_vone.rearrange("p (h o) -> p h o", o=1))

    xkT_r = xkT.rearrange("(kc p) l -> p kc l", p=128)
    xqT_r = xqT.rearrange("(kc p) l -> p kc l", p=128)
    xvT_r = xvT.rearrange("(kc p) l -> p kc l", p=128)
    wqT_r = wqT.rearrange("(kc p) m -> p kc m", p=128)
    wkT_r = wkT.rearrange("(kc p) m -> p kc m", p=128)
    wvT_r = wvT.rearrange("(kc p) m -> p kc m", p=128)
    woT_r = woT.rearrange("(kc p) m -> p kc m", p=128)

    nc.sync.dma_start(out=xk_sb[:], in_=xkT_r[:])
    nc.sync.dma_start(out=xq_sb[:], in_=xqT_r[:])
    nc.sync.dma_start(out=wv_sb[:], in_=wvT_r[:])

    # ---- phase 1a: value projection -------------------------------------
    with ExitStack() as vctx:
        pxv = vctx.enter_context(tc.tile_pool(name="pxv", bufs=3))
        ppv = vctx.enter_context(tc.tile_pool(name="ppv", bufs=4, space="PSUM"))
        for m in range(NJ):
            xb = pxv.tile([128, KC, 128], BF16, tag="xv")
            nc.sync.dma_start(out=xb[:], in_=xvT_r[:, :, m * 128:(m + 1) * 128])
            va = vpa[m].rearrange("p (h c) -> p h c", c=65)
            for n in range(2):
                nsl = slice(n * 512, (n + 1) * 512)
                ps = ppv.tile([128, 512], F32, tag="pv")
                nc.tensor.matmul(ps[:], ones1[0:1, :], bv_sb[0:1, nsl],
                                 start=True, stop=False)
                for kc in range(KC):
                    nc.tensor.matmul(ps[:], xb[:, kc, :], wv_sb[:, kc, nsl],
                                     start=False, stop=(kc == KC - 1))
                nc.vector.tensor_copy(va[:, 8 * n:8 * (n + 1), 0:64], ps[:])

    # ---- phase 1b + 2: k/q projections interleaved with attention -------
    # PSUM budget (8 banks): psAB 2 bufs x [128,1024] = 4, psO 2 tags x 1
    # buf x [65,512] = 2, ppk 2 bufs x [128,512] = 2.
    attn_ctx = ExitStack()
    pw = attn_ctx.enter_context(tc.tile_pool(name="pw", bufs=2))
    ppk = attn_ctx.enter_context(tc.tile_pool(name="ppk", bufs=2, space="PSUM"))
    psA = attn_ctx.enter_context(tc.tile_pool(name="psA", bufs=2, space="PSUM"))
    psO = attn_ctx.enter_context(tc.tile_pool(name="psO", bufs=1, space="PSUM"))
    pe = attn_ctx.enter_context(tc.tile_pool(name="pe", bufs=3))
    prc = attn_ctx.enter_context(tc.tile_pool(name="prc", bufs=2))

    wk_tiles = {}

    def emit_kq_load(p):
        wkb = pw.tile([128, KC, 128], BF16, tag="wk")
        nc.sync.dma_start(out=wkb[:], in_=wkT_r[:, :, p * 128:(p + 1) * 128])
        wqb = pw.tile([128, KC, 128], BF16, tag="wq")
        nc.sync.dma_start(out=wqb[:], in_=wqT_r[:, :, p * 128:(p + 1) * 128])
        wk_tiles[p] = (wkb, wqb)

    def emit_kq_chunk(p, c):
        # chunks 0-3: kpT[p] 512-col chunk c; chunks 4-5: qpT[p] chunk c-4
        wkb, wqb = wk_tiles[p]
        if c < 4:
            wb, x_sb, dst, bias, cc = wkb, xk_sb, kpT[p], bk_sb, c
        else:
            wb, x_sb, dst, bias, cc = wqb, xq_sb, qpT[p], bq_sb, c - 4
        csl = slice(cc * 512, (cc + 1) * 512)
        ps = ppk.tile([128, 512], F32, tag="pk")
        for kc in range(KC):
            nc.tensor.matmul(ps[:], wb[:, kc, :], x_sb[:, kc, csl],
                             start=(kc == 0), stop=(kc == KC - 1))
        nc.vector.tensor_scalar_add(dst[:, csl], ps[:], bias[:, p:p + 1])

    emit_kq_load(0)
    for c in range(6):
        emit_kq_chunk(0, c)

    # flattened item stream: one item = one key j-chunk of one (pair, ic)
    # block. Scores for both heads of the pair sit side by side in one
    # [128, 1024] psum tile so a single exp op covers them.
    items = [(p, ic, j)
             for p in range(NPAIR) for ic in range(NI) for j in range(NJ)]
    s_t, e_t = {}, {}
    oz = {}

    def emit_S(k):
        p, ic, j = items[k]
        isl = slice(ic * 512, (ic + 1) * 512)
        jsl = slice(j * 128, (j + 1) * 128)
        s = psA.tile([128, 1024], F32, tag="s")
        nc.tensor.matmul(s[:, 0:512], kpT[p][0:64, jsl], qpT[p][0:64, isl],
                         tile_position=(0, 0))
        nc.tensor.matmul(s[:, 512:1024], kpT[p][64:128, jsl],
                         qpT[p][64:128, isl], tile_position=(64, 0))
        s_t[k] = s

    def emit_exp(k):
        s = s_t.pop(k)
        e = pe.tile([128, 1024], BF16, tag="e")
        nc.scalar.activation(e[:], s[:], EXPF)
        e_t[k] = e

    def emit_PV(k):
        p, ic, j = items[k]
        hA, hB = 2 * p, 2 * p + 1
        if j == 0:
            ozA = psO.tile([65, 512], F32, tag="oa", name=f"ozA{p}_{ic}")
            ozB = psO.tile([65, 512], F32, tag="ob", name=f"ozB{p}_{ic}")
            oz[(p, ic)] = (ozA, ozB)
        ozA, ozB = oz[(p, ic)]
        e = e_t.pop(k)
        nc.tensor.matmul(ozA[:, :], vpa[j][:, hA * 65:(hA + 1) * 65],
                         e[:, 0:512], start=(j == 0), stop=(j == NJ - 1))
        nc.tensor.matmul(ozB[:, :], vpa[j][:, hB * 65:(hB + 1) * 65],
                         e[:, 512:1024], start=(j == 0), stop=(j == NJ - 1))

    def emit_block_end(k):
        p, ic, j = items[k]
        ozA, ozB = oz.pop((p, ic))
        isl = slice(ic * 512, (ic + 1) * 512)
        # unnormalized head outputs -> SBUF (normalized in place later)
        nc.vector.tensor_copy(outU[p][0:64, isl], ozA[0:64, :])
        nc.vector.tensor_copy(outU[p][64:128, isl], ozB[0:64, :])
        # Z rows (psum partition 64) -> zall rows via staging + DMA
        zst = prc.tile([65, 1024], F32, tag="zs")
        nc.vector.tensor_copy(zst[64:65, 0:512], ozA[64:65, :])
        nc.vector.tensor_copy(zst[64:65, 512:1024], ozB[64:65, :])
        r0 = 4 * p + 2 * ic
        nc.sync.dma_start(out=zall[r0:r0 + 1, :], in_=zst[64:65, 0:512])
        nc.sync.dma_start(out=zall[r0 + 1:r0 + 2, :], in_=zst[64:65, 512:1024])

    # software-pipelined emission: S runs 2 items ahead of exp/PV
    NIT = len(items)
    emit_S(0)
    emit_exp(0)
    emit_S(1)
    emit_exp(1)
    kq_work = [(p, c) for p in range(1, NPAIR) for c in range(-1, 6)]
    kq_i = 0
    for k in range(NIT):
        emit_PV(k)
        if k == NIT - 1 or items[k + 1][2] == 0:
            emit_block_end(k)
        # interleave next pair's projection work into this pair's stream
        p_cur, ic_cur, j_cur = items[k]
        slot = ic_cur * NJ + j_cur
        if slot % 4 == 0:
            while kq_i < len(kq_work) and kq_work[kq_i][0] == p_cur + 1:
                pp_, cc_ = kq_work[kq_i]
                if cc_ < 0:
                    emit_kq_load(pp_)
                else:
                    emit_kq_chunk(pp_, cc_)
                kq_i += 1
                if cc_ >= 0:
                    break
        if k + 2 < NIT:
            emit_S(k + 2)
            emit_exp(k + 2)
    attn_ctx.close()

    # ---- phase 2b: batched softmax normalization ------------------------
    with ExitStack() as nctx:
        pn = nctx.enter_context(tc.tile_pool(name="pn", bufs=1))
        psN = nctx.enter_context(tc.tile_pool(name="psN", bufs=2, space="PSUM"))
        rz = pn.tile([64, 512], F32R, name="rz")
        with nc.allow_low_precision(reason="fp32r rounding of 1/Z"):
            nc.vector.reciprocal(rz[:], zall[:])
        for p in range(NPAIR):
            for ic in range(NI):
                r0 = 4 * p + 2 * ic
                isl = slice(ic * 512, (ic + 1) * 512)
                rzb = psN.tile([128, 512], F32, tag="rzb")
                nc.tensor.matmul(rzb[:, :],
                                 sel_sb[:, r0 * 64:(r0 + 2) * 64], rz[:])
                nc.vector.tensor_mul(outU[p][:, isl], outU[p][:, isl], rzb[:])

    # ---- phase 3: output projection -------------------------------------
    with ExitStack() as fin_ctx:
        fw = fin_ctx.enter_context(tc.tile_pool(name="fw", bufs=2))
        fs = fin_ctx.enter_context(tc.tile_pool(name="fs", bufs=3))
        pf = fin_ctx.enter_context(tc.tile_pool(name="pf", bufs=4, space="PSUM"))
        for n in range(D // 512):
            nsl = slice(n * 512, (n + 1) * 512)
            wob = fw.tile([128, KC, 512], BF16, tag="fw")
            nc.sync.dma_start(out=wob[:], in_=woT_r[:, :, nsl])
            for m in range(LQ // 128):
                msl = slice(m * 128, (m + 1) * 128)
                ps = pf.tile([128, 512], F32, tag="pf")
                nc.tensor.matmul(ps[:], ones1[0:1, :], bo_sb[0:1, nsl],
                                 start=True, stop=False)
                for kc in range(KC):
                    nc.tensor.matmul(ps[:], outU[kc][:, msl], wob[:, kc, :],
                                     start=False, stop=(kc == KC - 1))
                ost = fs.tile([128, 512], F32, tag="fs")
                nc.scalar.copy(ost[:], ps[:])
                nc.sync.dma_start(out=out[msl, nsl], in_=ost[:])


_NC_CACHE = None


def _get_program():
    global _NC_CACHE
    if _NC_CACHE is None:
        _NC_CACHE = build_program()
    return _NC_CACHE


def prep_in_maps(q, k, v, w_q, b_q, w_k, b_k, w_v, b_v, w_o, b_o):
    import ml_dtypes

    f = np.float32
    bf = ml_dtypes.bfloat16
    q, k, v = (np.asarray(t, f) for t in (q, k, v))
    scale = 1.0 / np.sqrt(DH)
    wqT = np.ascontiguousarray((np.asarray(w_q, f) * scale).T).astype(bf)
    wkT = np.ascontiguousarray(np.asarray(w_k, f).T).astype(bf)
    wvT = np.ascontiguousarray(np.asarray(w_v, f).T).astype(bf)
    woT = np.ascontiguousarray(np.asarray(w_o, f).T).astype(bf)
    bqc = np.ascontiguousarray((np.asarray(b_q, f) * scale).reshape(KC, 128).T)
    bkc = np.ascontiguousarray(np.asarray(b_k, f).reshape(KC, 128).T)
    bvr = np.asarray(b_v, f).reshape(1, D).astype(bf)
    bor = np.asarray(b_o, f).reshape(1, D).astype(bf)
    c_or = np.ones((1, 128), bf)
    c_vone = np.ones((128, NH), bf)
    c_sel = np.zeros((64, 64 * 4 * NPAIR), f)
    for r in range(4 * NPAIR):
        c_sel[r, r * 64:(r + 1) * 64] = 1.0
    in_maps = []
    for c in range(N_CORES):
        b, qh = c // 2, c % 2
        kTb = np.ascontiguousarray(k[b].T).astype(bf)
        vTb = np.ascontiguousarray(v[b].T).astype(bf)
        qTb = np.ascontiguousarray(q[b].T[:, qh * LQ:(qh + 1) * LQ]).astype(bf)
        in_maps.append({
            "xqT": qTb, "xkT": kTb, "xvT": vTb,
            "wqT": wqT, "wkT": wkT, "wvT": wvT, "woT": woT,
            "bqc": bqc, "bkc": bkc, "bvr": bvr, "bor": bor,
            "c_or": c_or, "c_vone": c_vone, "c_sel": c_sel,
        })
    return in_maps


def run(in_maps, trace=False, **kw):
    nc = _get_program()
    return run_bass_kernel_spmd(nc, in_maps, list(range(N_CORES)),
                                trace=trace, **kw)


def kernel(**inputs):
    in_maps = prep_in_maps(**inputs)
    res = run(in_maps)
    out = np.empty((B, L, D), np.float32)
    for c in range(N_CORES):
        b, qh = c // 2, c % 2
        out[b, qh * LQ:(qh + 1) * LQ, :] = res.results[c]["out"]
    return out


# revision 21
# speedup vs baseline: 2.4700x; 1.0934x over previous
"""Multi-head attention (B=4, L=2048, D=1024, H=16) on 8 Trainium2 NeuronCores.

Sharding: core c = (batch b = c//2, query-half qh = c%2). Each core computes
all 16 heads for its 1024 query rows against the full 2048 keys/values of its
batch. Fully SPMD, no collectives.

v2: all-bf16 data path (fp32 psum), software-pipelined attention emission
(S matmuls run two groups ahead of exp/PV so the PE never idles on the
activation engine), k/q projections for pair p+1 interleaved into pair p's
attention stream, deferred batched softmax normalization (no per-block
reciprocal chain), and SBUF-resident activations end-to-end (x, w, outT all
loaded once; no DRAM scratch roundtrip).

Per-core pipeline:
  1. vp[j,16x(64+1)] value projection + ones col (PV also yields Z row 64)
  2. kpT[p][128,2048], qpT[p][128,1024] per head-pair p (pair 0 upfront,
     pairs 1-7 streamed inside the attention loop)
  3. per (pair, 512-query block), 8 groups of 2 key chunks:
       S^T = kpT^T qpT (row-packed head pairs, K=64) -> psum
       P = exp(S^T) bf16 (ACT)  ->  outU += vp^T P, Z += ones^T P (PE)
     Z rows DMA-gathered into zall[32,512] (async, off critical path)
  4. rz = 1/zall (one DVE op), per (pair,ic): 2 selector matmuls broadcast
     rz across 64 partitions, one DVE mul normalizes outU in place
  5. out[l,o] = outU^T woT + bo streamed to DRAM
"""

import sys

if "/opt/trn_rl_repo" not in sys.path:
    sys.path.insert(0, "/opt/trn_rl_repo")

import numpy as np

import concourse.bacc as bacc
import concourse.tile as tile
from concourse import mybir
from concourse.bass_utils import run_bass_kernel_spmd

N_CORES = 8
B, L, D = 4, 2048, 1024
NH, DH = 16, 64          # heads, head dim
LQ = L // 2              # query rows per core
F32 = mybir.dt.float32
F32R = mybir.dt.float32r
BF16 = mybir.dt.bfloat16

KC = D // 128            # 8 contraction chunks for projections
NJ = L // 128            # 16 key j-chunks
NI = LQ // 512           # 2 query i-chunks of 512
NPAIR = NH // 2          # 8 head pairs
EXPF = mybir.ActivationFunctionType.Exp


def build_program():
    nc = bacc.Bacc("TRN2", target_bir_lowering=False, debug=False,
                   num_devices=N_CORES)
    with tile.TileContext(nc) as tc:
        _emit(nc, tc)
    nc.compile()
    return nc


def _emit(nc, tc):
    from contextlib import ExitStack

    top = ExitStack()
    dram = top.enter_context(tc.tile_pool(name="dram", bufs=1, space="DRAM"))

    def din(shape, dt, name):
        return dram.tile(shape, dt, kind="ExternalInput", name=name,
                         uniquify=False)

    xqT = din([D, LQ], BF16, "xqT")
    xkT = din([D, L], BF16, "xkT")
    xvT = din([D, L], BF16, "xvT")
    wqT = din([D, D], BF16, "wqT")
    wkT = din([D, D], BF16, "wkT")
    wvT = din([D, D], BF16, "wvT")
    woT = din([D, D], BF16, "woT")
    bqc = din([128, KC], F32, "bqc")
    bkc = din([128, KC], F32, "bkc")
    bvr = din([1, D], BF16, "bvr")
    bor = din([1, D], BF16, "bor")
    c_or = din([1, 128], BF16, "c_or")
    c_sel = din([64, 64 * 4 * NPAIR], F32R, "c_sel")
    out = dram.tile([LQ, D], F32, kind="ExternalOutput", name="out",
                    uniquify=False)

    # ---- persistent SBUF -------------------------------------------------
    pers = top.enter_context(tc.tile_pool(name="pers", bufs=1))
    kpT = [pers.tile([128, L], BF16, name=f"kpT{m}") for m in range(NPAIR)]
    qpT = [pers.tile([128, LQ], BF16, name=f"qpT{m}") for m in range(NPAIR)]
    # vpa: per j-chunk, 16 heads x (64 value cols + 1 ones col)
    vpa = [pers.tile([128, NH * 65], BF16, name=f"vpa{m}") for m in range(NJ)]
    outU = [pers.tile([128, LQ], BF16, name=f"outU{m}") for m in range(NPAIR)]
    xk_sb = pers.tile([128, KC, L], BF16, name="xk_sb")
    xq_sb = pers.tile([128, KC, LQ], BF16, name="xq_sb")
    wv_sb = pers.tile([128, KC, D], BF16, name="wv_sb")
    ones1 = pers.tile([1, 128], BF16, name="ones1")
    # 64 partitions (rows 32+ pad: zall 1.0, sel 0) for valid 64x64 PE tiling
    sel_sb = pers.tile([64, 64 * 4 * NPAIR], F32R, name="sel_sb")
    zall = pers.tile([64, 512], F32, name="zall")
    nc.vector.memset(zall[4 * NPAIR:64, :], 1.0)
    bq_sb = pers.tile([128, KC], F32, name="bq_sb")
    bk_sb = pers.tile([128, KC], F32, name="bk_sb")
    bv_sb = pers.tile([1, D], BF16, name="bv_sb")
    bo_sb = pers.tile([1, D], BF16, name="bo_sb")

    xkT_r = xkT.rearrange("(kc p) l -> p kc l", p=128)
    xqT_r = xqT.rearrange("(kc p) l -> p kc l", p=128)
    xvT_r = xvT.rearrange("(kc p) l -> p kc l", p=128)
    wqT_r = wqT.rearrange("(kc p) m -> p kc m", p=128)
    wkT_r = wkT.rearrange("(kc p) m -> p kc m", p=128)
    wvT_r = wvT.rearrange("(kc p) m -> p kc m", p=128)
    woT_r = woT.rearrange("(kc p) m -> p kc m", p=128)

    # vp-phase inputs first so the PE can start immediately
    nc.sync.dma_start(out=wv_sb[:], in_=wvT_r[:])
    nc.sync.dma_start(out=bv_sb[:], in_=bvr[:])
    nc.sync.dma_start(out=ones1[:], in_=c_or[:])
    for m in range(NJ):
        nc.vector.memset(
            vpa[m].rearrange("p (h c) -> p h c", c=65)[:, :, 64:65], 1.0)

    # ---- phase 1a: value projection -------------------------------------
    with ExitStack() as vctx:
        pxv = vctx.enter_context(tc.tile_pool(name="pxv", bufs=3))
        ppv = vctx.enter_context(tc.tile_pool(name="ppv", bufs=4, space="PSUM"))
        for m in range(NJ):
            xb = pxv.tile([128, KC, 128], BF16, tag="xv")
            nc.sync.dma_start(out=xb[:], in_=xvT_r[:, :, m * 128:(m + 1) * 128])
            va = vpa[m].rearrange("p (h c) -> p h c", c=65)
            for n in range(2):
                nsl = slice(n * 512, (n + 1) * 512)
                ps = ppv.tile([128, 512], F32, tag="pv")
                nc.tensor.matmul(ps[:], ones1[0:1, :], bv_sb[0:1, nsl],
                                 start=True, stop=False)
                for kc in range(KC):
                    nc.tensor.matmul(ps[:], xb[:, kc, :], wv_sb[:, kc, nsl],
                                     start=False, stop=(kc == KC - 1))
                nc.vector.tensor_copy(va[:, 8 * n:8 * (n + 1), 0:64], ps[:])

    # remaining inputs land while vp runs; chunked so kq waits per-slice
    for c in range(4):
        nc.sync.dma_start(out=xk_sb[:, :, c * 512:(c + 1) * 512],
                          in_=xkT_r[:, :, c * 512:(c + 1) * 512])
    for c in range(2):
        nc.sync.dma_start(out=xq_sb[:, :, c * 512:(c + 1) * 512],
                          in_=xqT_r[:, :, c * 512:(c + 1) * 512])
    nc.sync.dma_start(out=sel_sb[:], in_=c_sel[:])
    nc.sync.dma_start(out=bq_sb[:], in_=bqc[:])
    nc.sync.dma_start(out=bk_sb[:], in_=bkc[:])
    nc.sync.dma_start(out=bo_sb[:], in_=bor[:])

    # ---- phase 1b + 2: k/q projections interleaved with attention -------
    # PSUM budget (8 banks): psAB 2 bufs x [128,1024] = 4, psO 2 tags x 1
    # buf x [65,512] = 2, ppk 2 bufs x [128,512] = 2.
    attn_ctx = ExitStack()
    pw = attn_ctx.enter_context(tc.tile_pool(name="pw", bufs=2))
    ppk = attn_ctx.enter_context(tc.tile_pool(name="ppk", bufs=2, space="PSUM"))
    psA = attn_ctx.enter_context(tc.tile_pool(name="psA", bufs=2, space="PSUM"))
    psO = attn_ctx.enter_context(tc.tile_pool(name="psO", bufs=1, space="PSUM"))
    pe = attn_ctx.enter_context(tc.tile_pool(name="pe", bufs=3))
    prc = attn_ctx.enter_context(tc.tile_pool(name="prc", bufs=2))

    wk_tiles = {}

    def emit_kq_load(p):
        wkb = pw.tile([128, KC, 128], BF16, tag="wk")
        nc.sync.dma_start(out=wkb[:], in_=wkT_r[:, :, p * 128:(p + 1) * 128])
        wqb = pw.tile([128, KC, 128], BF16, tag="wq")
        nc.sync.dma_start(out=wqb[:], in_=wqT_r[:, :, p * 128:(p + 1) * 128])
        wk_tiles[p] = (wkb, wqb)

    kq_ps = {}

    def emit_kq_chunk(p, c, half=None):
        # chunks 0-3: kpT[p] 512-col chunk c; chunks 4-5: qpT[p] chunk c-4
        # half=0/1 emits only the lower/upper kc contraction half (so the
        # matmul burst can be split across two pipeline slots)
        wkb, wqb = wk_tiles[p]
        if c < 4:
            wb, x_sb, dst, bias, cc = wkb, xk_sb, kpT[p], bk_sb, c
        else:
            wb, x_sb, dst, bias, cc = wqb, xq_sb, qpT[p], bq_sb, c - 4
        csl = slice(cc * 512, (cc + 1) * 512)
        if half in (None, 0):
            ps = ppk.tile([128, 512], F32, tag="pk", name=f"pk{p}_{c}")
            kq_ps[(p, c)] = ps
        else:
            ps = kq_ps.pop((p, c))
        kcs = range(KC) if half is None else range(half * KC // 2,
                                                  (half + 1) * KC // 2)
        for kc in kcs:
            nc.tensor.matmul(ps[:], wb[:, kc, :], x_sb[:, kc, csl],
                             start=(kc == 0), stop=(kc == KC - 1))
        if half in (None, 1):
            nc.vector.tensor_scalar_add(dst[:, csl], ps[:], bias[:, p:p + 1])

    emit_kq_load(0)
    for c in range(6):
        emit_kq_chunk(0, c)

    # flattened item stream: one item = one key j-chunk of one (pair, ic)
    # block. Scores for both heads of the pair sit side by side in one
    # [128, 1024] psum tile so a single exp op covers them.
    items = [(p, ic, j)
             for p in range(NPAIR) for ic in range(NI) for j in range(NJ)]
    s_t, e_t = {}, {}
    oz = {}

    def emit_S(k):
        p, ic, j = items[k]
        isl = slice(ic * 512, (ic + 1) * 512)
        jsl = slice(j * 128, (j + 1) * 128)
        s = psA.tile([128, 1024], F32, tag="s")
        nc.tensor.matmul(s[:, 0:512], kpT[p][0:64, jsl], qpT[p][0:64, isl],
                         tile_position=(0, 0))
        nc.tensor.matmul(s[:, 512:1024], kpT[p][64:128, jsl],
                         qpT[p][64:128, isl], tile_position=(64, 0))
        s_t[k] = s

    def emit_exp(k):
        s = s_t.pop(k)
        e = pe.tile([128, 1024], BF16, tag="e")
        nc.scalar.activation(e[:], s[:], EXPF)
        e_t[k] = e

    def emit_PV(k):
        p, ic, j = items[k]
        hA, hB = 2 * p, 2 * p + 1
        if j == 0:
            ozA = psO.tile([65, 512], F32, tag="oa", name=f"ozA{p}_{ic}")
            ozB = psO.tile([65, 512], F32, tag="ob", name=f"ozB{p}_{ic}")
            oz[(p, ic)] = (ozA, ozB)
        ozA, ozB = oz[(p, ic)]
        e = e_t.pop(k)
        nc.tensor.matmul(ozA[:, :], vpa[j][:, hA * 65:(hA + 1) * 65],
                         e[:, 0:512], start=(j == 0), stop=(j == NJ - 1))
        nc.tensor.matmul(ozB[:, :], vpa[j][:, hB * 65:(hB + 1) * 65],
                         e[:, 512:1024], start=(j == 0), stop=(j == NJ - 1))

    def emit_block_end(k):
        p, ic, j = items[k]
        ozA, ozB = oz.pop((p, ic))
        isl = slice(ic * 512, (ic + 1) * 512)
        # unnormalized head outputs -> SBUF (normalized in place later)
        nc.vector.tensor_copy(outU[p][0:64, isl], ozA[0:64, :])
        nc.vector.tensor_copy(outU[p][64:128, isl], ozB[0:64, :])
        # Z rows (psum partition 64) -> zall rows via staging + DMA
        zst = prc.tile([65, 1024], F32, tag="zs")
        nc.vector.tensor_copy(zst[64:65, 0:512], ozA[64:65, :])
        nc.vector.tensor_copy(zst[64:65, 512:1024], ozB[64:65, :])
        r0 = 4 * p + 2 * ic
        nc.sync.dma_start(out=zall[r0:r0 + 1, :], in_=zst[64:65, 0:512])
        nc.sync.dma_start(out=zall[r0 + 1:r0 + 2, :], in_=zst[64:65, 512:1024])

    # software-pipelined emission: S runs 2 items ahead of exp/PV
    NIT = len(items)
    emit_S(0)
    emit_exp(0)
    emit_S(1)
    emit_exp(1)
    kq_work = []
    for p in range(1, NPAIR):
        kq_work.append((p, -1, None))
        for c in range(6):
            kq_work.append((p, c, 0))
            kq_work.append((p, c, 1))
    kq_i = 0
    for k in range(NIT):
        emit_PV(k)
        if k == NIT - 1 or items[k + 1][2] == 0:
            emit_block_end(k)
        # interleave next pair's projection work into this pair's stream,
        # half a contraction chunk per slot to keep the PE queue smooth
        p_cur, ic_cur, j_cur = items[k]
        slot = ic_cur * NJ + j_cur
        if slot % 2 == 0:
            while kq_i < len(kq_work) and kq_work[kq_i][0] == p_cur + 1:
                pp_, cc_, hh_ = kq_work[kq_i]
                if cc_ < 0:
                    emit_kq_load(pp_)
                else:
                    emit_kq_chunk(pp_, cc_, hh_)
                kq_i += 1
                if cc_ >= 0:
                    break
        if k + 2 < NIT:
            emit_S(k + 2)
            emit_exp(k + 2)
    attn_ctx.close()

    # ---- phase 2b: batched softmax normalization ------------------------
    with ExitStack() as nctx:
        pn = nctx.enter_context(tc.tile_pool(name="pn", bufs=1))
        psN = nctx.enter_context(tc.tile_pool(name="psN", bufs=2, space="PSUM"))
        rz = pn.tile([64, 512], F32R, name="rz")
        with nc.allow_low_precision(reason="fp32r rounding of 1/Z"):
            nc.vector.reciprocal(rz[:], zall[:])
        # ic=0 first so phase 3's first half can start while ic=1 normalizes
        for ic in range(NI):
            for p in range(NPAIR):
                r0 = 4 * p + 2 * ic
                isl = slice(ic * 512, (ic + 1) * 512)
                rzb = psN.tile([128, 512], F32, tag="rzb")
                nc.tensor.matmul(rzb[:, :],
                                 sel_sb[:, r0 * 64:(r0 + 2) * 64], rz[:])
                nc.vector.tensor_mul(outU[p][:, isl], outU[p][:, isl], rzb[:])

    # ---- phase 3: output projection -------------------------------------
    with ExitStack() as fin_ctx:
        fw = fin_ctx.enter_context(tc.tile_pool(name="fw", bufs=2))
        fs = fin_ctx.enter_context(tc.tile_pool(name="fs", bufs=3))
        pf = fin_ctx.enter_context(tc.tile_pool(name="pf", bufs=4, space="PSUM"))
        for n in range(D // 512):
            nsl = slice(n * 512, (n + 1) * 512)
            wob = fw.tile([128, KC, 512], BF16, tag="fw")
            nc.sync.dma_start(out=wob[:], in_=woT_r[:, :, nsl])
            for m in range(LQ // 128):
                msl = slice(m * 128, (m + 1) * 128)
                ps = pf.tile([128, 512], F32, tag="pf")
                nc.tensor.matmul(ps[:], ones1[0:1, :], bo_sb[0:1, nsl],
                                 start=True, stop=False)
                for kc in range(KC):
                    nc.tensor.matmul(ps[:], outU[kc][:, msl], wob[:, kc, :],
                                     start=False, stop=(kc == KC - 1))
                ost = fs.tile([128, 512], F32, tag="fs")
                nc.vector.tensor_copy(ost[:], ps[:])
                nc.sync.dma_start(out=out[msl, nsl], in_=ost[:])


_NC_CACHE = None


def _get_program():
    global _NC_CACHE
    if _NC_CACHE is None:
        _NC_CACHE = build_program()
    return _NC_CACHE


def prep_in_maps(q, k, v, w_q, b_q, w_k, b_k, w_v, b_v, w_o, b_o):
    import ml_dtypes

    f = np.float32
    bf = ml_dtypes.bfloat16
    q, k, v = (np.asarray(t, f) for t in (q, k, v))
    scale = 1.0 / np.sqrt(DH)
    wqT = np.ascontiguousarray((np.asarray(w_q, f) * scale).T).astype(bf)
    wkT = np.ascontiguousarray(np.asarray(w_k, f).T).astype(bf)
    wvT = np.ascontiguousarray(np.asarray(w_v, f).T).astype(bf)
    woT = np.ascontiguousarray(np.asarray(w_o, f).T).astype(bf)
    bqc = np.ascontiguousarray((np.asarray(b_q, f) * scale).reshape(KC, 128).T)
    bkc = np.ascontiguousarray(np.asarray(b_k, f).reshape(KC, 128).T)
    bvr = np.asarray(b_v, f).reshape(1, D).astype(bf)
    bor = np.asarray(b_o, f).reshape(1, D).astype(bf)
    c_or = np.ones((1, 128), bf)
    c_sel = np.zeros((64, 64 * 4 * NPAIR), f)
    for r in range(4 * NPAIR):
        c_sel[r, r * 64:(r + 1) * 64] = 1.0
    in_maps = []
    for c in range(N_CORES):
        b, qh = c // 2, c % 2
        kTb = np.ascontiguousarray(k[b].T).astype(bf)
        vTb = np.ascontiguousarray(v[b].T).astype(bf)
        qTb = np.ascontiguousarray(q[b].T[:, qh * LQ:(qh + 1) * LQ]).astype(bf)
        in_maps.append({
            "xqT": qTb, "xkT": kTb, "xvT": vTb,
            "wqT": wqT, "wkT": wkT, "wvT": wvT, "woT": woT,
            "bqc": bqc, "bkc": bkc, "bvr": bvr, "bor": bor,
            "c_or": c_or, "c_sel": c_sel,
        })
    return in_maps


def run(in_maps, trace=False, **kw):
    nc = _get_program()
    return run_bass_kernel_spmd(nc, in_maps, list(range(N_CORES)),
                                trace=trace, **kw)


def kernel(**inputs):
    in_maps = prep_in_maps(**inputs)
    res = run(in_maps)
    out = np.empty((B, L, D), np.float32)
    for c in range(N_CORES):
        b, qh = c // 2, c % 2
        out[b, qh * LQ:(qh + 1) * LQ, :] = res.results[c]["out"]
    return out
